# revision 20
# baseline (speedup 1.0000x reference)
"""Trainium2 Bass kernel for nn_DeTrCrossAttention (self-contained).

Math (per batch b, derived from the reference's chained permutes):
  64 effective heads indexed by d=0..63, head dim 16 (the h axis).
  Q'[s, d*16+h] = (x_b @ Wq.T + bq)[s, h*64+d]
  K'[s, d*16+h] = (mem_b @ Wkv.T + bkv)[s, h*128+d]
  V'[s, d*16+h] = (mem_b @ Wkv.T)[s, h*128+64+d]      (bv folded into bd_eff)
  S_d = Q'_d @ K'_d.T / 8 ; P_d = softmax(S_d) ; ctx_d = P_d @ V'_d
  out = ctx @ Wd.T + bd_eff,  bd_eff = bd + Wd @ bv'

Sharding: pure data-parallel, one batch element per NeuronCore (B=8, 8 cores).
All matmuls use float32r (full PE stream rate at N>=256, fp32 storage).
Softmax runs in S^T layout [Sk partitions, Sq free]; denominators come from a
ones-column appended to V' (col-tiled M=17 ctx matmuls); no max subtraction
(|S/8| <= ~3 for these input scales, exp is safe in fp32).
"""

import os
import sys

import numpy as np

for _p in ("/opt/trn_rl_repo", os.path.expanduser("~/.axon_site/_ro/trn_rl_repo")):
    if os.path.isdir(_p) and _p not in sys.path:
        sys.path.insert(0, _p)

import concourse.bass as bass
import concourse.tile as tile
from concourse import bacc, mybir

HIDDEN = 1024
NHEAD = 16          # reference NUM_HEADS (becomes the contraction axis)
DH = 64             # reference HEAD_SIZE (becomes the effective head axis)
SQ, SK, B = 256, 1024, 8
E = HIDDEN
ND = DH             # 64 effective heads
HD = NHEAD          # 16 = effective head dim
F32 = mybir.dt.float32
F32R = mybir.dt.float32r


def build_nc():
    nc = bacc.Bacc("TRN2", target_bir_lowering=False)

    xT = nc.dram_tensor("xT", [E, SQ], F32R, kind="ExternalInput")
    mT = nc.dram_tensor("mT", [E, SK], F32R, kind="ExternalInput")
    wqT = nc.dram_tensor("wqT", [E, E], F32R, kind="ExternalInput")
    wkT = nc.dram_tensor("wkT", [E, E], F32R, kind="ExternalInput")
    wvT = nc.dram_tensor("wvT", [E, ND * 17], F32R, kind="ExternalInput")
    wdT = nc.dram_tensor("wdT", [2 * E, E], F32R, kind="ExternalInput")
    bq_d = nc.dram_tensor("bq", [1, E], F32R, kind="ExternalInput")
    bk_d = nc.dram_tensor("bk", [1, E], F32R, kind="ExternalInput")
    bva_d = nc.dram_tensor("bva", [1, ND * 17], F32R, kind="ExternalInput")
    bd_d = nc.dram_tensor("bd", [1, E], F32R, kind="ExternalInput")
    qmask_d = nc.dram_tensor("qmask", [128, 2], F32, kind="ExternalInput")
    ones_d = nc.dram_tensor("onesr", [1, E], F32R, kind="ExternalInput")
    zeros_d = nc.dram_tensor("zerosr", [1, SQ], F32R, kind="ExternalInput")
    out_d = nc.dram_tensor("out", [SQ, E], F32, kind="ExternalOutput")

    VCH = [(0, 384), (384, 384), (768, 320)]  # V'aug N-chunks (all >=256)

    with tile.TileContext(nc) as tc:
        from contextlib import ExitStack

        with ExitStack() as ctx:
            consts = ctx.enter_context(tc.tile_pool(name="consts", bufs=1))
            qab_p = ctx.enter_context(tc.tile_pool(name="qab", bufs=1))
            kt_p = ctx.enter_context(tc.tile_pool(name="ktflat", bufs=1))
            va_p = ctx.enter_context(tc.tile_pool(name="vaug", bufs=1))
            cx_p = ctx.enter_context(tc.tile_pool(name="ctxT", bufs=1))

            ones = consts.tile([1, E], F32R, tag="ones")
            nc.sync.dma_start(ones[:], ones_d[:])
            bq_s = consts.tile([1, E], F32R, tag="bq")
            bk_s = consts.tile([1, E], F32R, tag="bk")
            bva_s = consts.tile([1, ND * 17], F32R, tag="bva")
            bd_s = consts.tile([1, E], F32R, tag="bd")
            qmask_s = consts.tile([128, 2], F32, tag="qmask")
            nc.sync.dma_start(bq_s[:], bq_d[:])
            nc.sync.dma_start(bk_s[:], bk_d[:])
            nc.sync.dma_start(bva_s[:], bva_d[:])
            nc.sync.dma_start(bd_s[:], bd_d[:])
            nc.sync.dma_start(qmask_s[:], qmask_d[:])

            # persistent attention operands
            qa = [qab_p.tile([128, SQ], F32R, tag=f"qa{m}", name=f"qa{m}") for m in range(8)]
            qb = [qab_p.tile([128, SQ], F32R, tag=f"qb{m}", name=f"qb{m}") for m in range(8)]
            kt = [kt_p.tile([128, SK], F32R, tag=f"kt{m}", name=f"kt{m}") for m in range(8)]
            va = [va_p.tile([128, ND * 17], F32R, tag=f"va{m}", name=f"va{m}") for m in range(8)]
            cxT = [cx_p.tile([128, SQ], F32R, tag=f"cx{g}", name=f"cx{g}") for g in range(16)]

            # ---------------- phase 1: projections ----------------
            with ExitStack() as ph1:
                xt_p = ph1.enter_context(tc.tile_pool(name="xt", bufs=1))
                mt_p = ph1.enter_context(tc.tile_pool(name="mt", bufs=1))
                wv_p = ph1.enter_context(tc.tile_pool(name="wv", bufs=1))
                wstr = ph1.enter_context(tc.tile_pool(name="wstr", bufs=4))
                pps = ph1.enter_context(
                    tc.tile_pool(name="pps", bufs=2, space="PSUM")
                )

                xt = [xt_p.tile([128, SQ], F32R, tag=f"xt{k}", name=f"xt{k}") for k in range(8)]
                mt = [mt_p.tile([128, SK], F32R, tag=f"mt{k}", name=f"mt{k}") for k in range(8)]
                for k in range(8):
                    nc.sync.dma_start(xt[k][:], xT[128 * k : 128 * (k + 1), :])
                    nc.sync.dma_start(mt[k][:], mT[128 * k : 128 * (k + 1), :])

                # V'aug = mT.T @ wvT  (+ bias row: only ones-columns nonzero)
                wv = [wv_p.tile([128, ND * 17], F32R, tag=f"wv{k}", name=f"wv{k}") for k in range(8)]
                for k in range(8):
                    nc.sync.dma_start(wv[k][:], wvT[128 * k : 128 * (k + 1), :])
                for mk in range(8):
                    for n0, nw in VCH:
                        ps = pps.tile([128, 384], F32, tag="vps")
                        for k in range(8):
                            nc.tensor.matmul(
                                ps[:, :nw],
                                (mt[k][:, 128 * mk : 128 * (mk + 1)]),
                                (wv[k][:, n0 : n0 + nw]),
                                start=(k == 0),
                                stop=False,
                            )
                        nc.tensor.matmul(
                            ps[:, :nw],
                            (ones[0:1, 0:128]),
                            (bva_s[0:1, n0 : n0 + nw]),
                            start=False,
                            stop=True,
                        )
                        nc.vector.tensor_copy(va[mk][:, n0 : n0 + nw], ps[:, :nw])

                # K'T = wkT.T @ mT   -> kt[m] (flat, rows 16j+h)
                for m in range(8):
                    for n in range(2):
                        ps = pps.tile([128, 512], F32, tag="kps")
                        for k in range(8):
                            wt = wstr.tile([128, 128], F32R, tag="wkt")
                            nc.sync.dma_start(
                                wt[:],
                                wkT[128 * k : 128 * (k + 1), 128 * m : 128 * (m + 1)],
                            )
                            nc.tensor.matmul(
                                ps[:],
                                (wt[:]),
                                (mt[k][:, 512 * n : 512 * (n + 1)]),
                                start=(k == 0),
                                stop=False,
                            )
                        nc.tensor.matmul(
                            ps[:],
                            (bk_s[0:1, 128 * m : 128 * (m + 1)]),
                            (ones[0:1, 0:512]),
                            start=False,
                            stop=True,
                        )
                        nc.vector.tensor_copy(
                            kt[m][:, 512 * n : 512 * (n + 1)], ps[:]
                        )

                # Q'T = wqT.T @ xT -> split even/odd 16-row slices into qa/qb
                for m in range(8):
                    ps = pps.tile([128, SQ], F32, tag="qps")
                    for k in range(8):
                        wt = wstr.tile([128, 128], F32R, tag="wqt")
                        nc.sync.dma_start(
                            wt[:],
                            wqT[128 * k : 128 * (k + 1), 128 * m : 128 * (m + 1)],
                        )
                        nc.tensor.matmul(
                            ps[:], (wt[:]), (xt[k][:]), start=(k == 0), stop=False
                        )
                    nc.tensor.matmul(
                        ps[:],
                        (bq_s[0:1, 128 * m : 128 * (m + 1)]),
                        (ones[0:1, 0:256]),
                        start=False,
                        stop=True,
                    )
                    # full-width copies then per-partition masks (even/odd
                    # 16-row slices; engine APs need 32-aligned bases)
                    nc.vector.tensor_scalar_mul(qa[m][:], ps[:], qmask_s[:, 0:1])
                    nc.vector.tensor_scalar_mul(qb[m][:], ps[:], qmask_s[:, 1:2])

            # ---------------- phase 2: attention ----------------
            with ExitStack() as ph2:
                pt_p = ph2.enter_context(tc.tile_pool(name="pt", bufs=3))
                r_p = ph2.enter_context(tc.tile_pool(name="rt", bufs=3))
                st_p = ph2.enter_context(
                    tc.tile_pool(name="st", bufs=3, space="PSUM")
                )
                cxps = ph2.enter_context(
                    tc.tile_pool(name="cxps", bufs=2, space="PSUM")
                )

                # zero the never-written pad rows of each ctxT slot once
                for g in range(16):
                    for jj in range(4):
                        nc.sync.dma_start(
                            cxT[g][32 * jj + 17 : 32 * jj + 32, :],
                            zeros_d[0:1, :].unsqueeze(1).broadcast_to([1, 15, SQ]),
                        )

                for d in range(ND):
                    g, jj = d // 4, d % 4
                    m, j8 = d // 8, d % 8
                    p = j8 // 2
                    qsrc = qa[m] if j8 % 2 == 0 else qb[m]
                    pt = pt_p.tile([128, 2048], F32R, tag="pt", name="pt")
                    for half in range(2):
                        st = st_p.tile([128, 1024], F32, tag="st", name="st")
                        for i in range(4):
                            kb = 4 * half + i
                            nc.tensor.matmul(
                                st[:, 256 * i : 256 * (i + 1)],
                                kt[m][32 * p : 32 * (p + 1),
                                      128 * kb : 128 * (kb + 1)],
                                qsrc[32 * p : 32 * (p + 1), :],
                                start=True,
                                stop=True,
                                tile_position=(32 * p, 0),
                            )
                        nc.scalar.activation(
                            pt[:, 1024 * half : 1024 * (half + 1)],
                            st[:],
                            mybir.ActivationFunctionType.Exp,
                            scale=0.125,
                        )
                    cps = cxps.tile([17, SQ], F32, tag="cps", name="cps")
                    for kb in range(8):
                        nc.tensor.matmul(
                            cps[:],
                            va[kb][:, 17 * d : 17 * d + 17],
                            pt[:, 256 * kb : 256 * (kb + 1)],
                            start=(kb == 0),
                            stop=(kb == 7),
                        )
                    # unnormalized ctx rows + denominator row -> ctxT slot
                    nc.vector.tensor_copy(cxT[g][32 * jj : 32 * jj + 17, :], cps[:])
                    if jj == 0:
                        rtd = r_p.tile([128, SQ], F32R, tag="rtd", name="rtd")
                        rtr = r_p.tile([128, SQ], F32R, tag="rtr", name="rtr")
                    lo, hi = 32 * jj, 32 * jj + 16
                    nc.sync.dma_start(
                        rtd[lo:hi, :],
                        cxT[g][hi : hi + 1, :]
                        .unsqueeze(1)
                        .broadcast_to([1, 16, SQ]),
                    )
                    with nc.allow_low_precision(reason="f32r recip of softmax denom"):
                        nc.vector.reciprocal(rtr[lo:hi, :], rtd[lo:hi, :])
                    nc.vector.tensor_mul(
                        cxT[g][lo:hi, :],
                        cxT[g][lo:hi, :],
                        rtr[lo:hi, :],
                    )

            # ---------------- phase 3: output projection ----------------
            with ExitStack() as ph3:
                wd_p = ph3.enter_context(tc.tile_pool(name="wd", bufs=1))
                ob_p = ph3.enter_context(tc.tile_pool(name="ob", bufs=2))
                ops = ph3.enter_context(
                    tc.tile_pool(name="ops", bufs=2, space="PSUM")
                )
                wd = [wd_p.tile([128, E], F32R, tag=f"wd{g}", name=f"wd{g}") for g in range(16)]
                for g in range(16):
                    nc.sync.dma_start(wd[g][:], wdT[128 * g : 128 * (g + 1), :])
                for qb_i in range(2):
                    for n in range(2):
                        ps = ops.tile([128, 512], F32, tag="ops")
                        for g in range(16):
                            nc.tensor.matmul(
                                ps[:],
                                (cxT[g][:, 128 * qb_i : 128 * (qb_i + 1)]),
                                (wd[g][:, 512 * n : 512 * (n + 1)]),
                                start=(g == 0),
                                stop=False,
                            )
                        nc.tensor.matmul(
                            ps[:],
                            (ones[0:1, 128 * qb_i : 128 * qb_i + 128]),
                            (bd_s[0:1, 512 * n : 512 * (n + 1)]),
                            start=False,
                            stop=True,
                        )
                        ob = ob_p.tile([128, 512], F32, tag="ob")
                        nc.vector.tensor_copy(ob[:], ps[:])
                        nc.sync.dma_start(
                            out_d[128 * qb_i : 128 * (qb_i + 1),
                                  512 * n : 512 * (n + 1)],
                            ob[:],
                        )

    nc.compile()
    return nc


def host_prep(x, memory, Wq, bq, Wkv, bkv, Wd, bd):
    """Pure-numpy layout transforms shared by all cores + per-core slices."""
    x = np.asarray(x, np.float32)
    memory = np.asarray(memory, np.float32)
    Wq = np.asarray(Wq, np.float32)
    bq = np.asarray(bq, np.float32)
    Wkv = np.asarray(Wkv, np.float32)
    bkv = np.asarray(bkv, np.float32)
    Wd = np.asarray(Wd, np.float32)
    bd = np.asarray(bd, np.float32)

    d_i = np.arange(ND)[:, None]  # 64
    h_i = np.arange(HD)[None, :]  # 16
    perm_q = (h_i * DH + d_i).reshape(-1)          # e' = d*16+h -> h*64+d
    perm_k = (h_i * 2 * DH + d_i).reshape(-1)      # -> h*128+d
    perm_v = (h_i * 2 * DH + DH + d_i).reshape(-1)  # -> h*128+64+d

    wqT = np.ascontiguousarray(Wq[perm_q].T)
    wkT = np.ascontiguousarray(Wkv[perm_k].T)
    bq_p = bq[perm_q].reshape(1, E)
    bk_p = bkv[perm_k].reshape(1, E)
    bv_p = bkv[perm_v]

    wvT = np.zeros((E, ND * 17), np.float32)
    wvT.reshape(E, ND, 17)[:, :, :16] = Wkv[perm_v].T.reshape(E, ND, HD)
    bva = np.zeros((1, ND * 17), np.float32)
    bva.reshape(ND, 17)[:, 16] = 1.0

    wdT = np.zeros((2 * E, E), np.float32)
    wdT.reshape(ND, 32, E)[:, :16, :] = Wd.T.reshape(ND, HD, E)
    bd_eff = (bd + Wd @ bv_p).reshape(1, E)

    qmask = np.zeros((128, 2), np.float32)
    rows = np.arange(128)
    qmask[(rows // 16) % 2 == 0, 0] = 1.0  # even 16-row slices -> qa
    qmask[(rows // 16) % 2 == 1, 1] = 1.0  # odd slices -> qb

    shared = dict(wqT=wqT, wkT=wkT, wvT=wvT, wdT=wdT,
                  bq=bq_p, bk=bk_p, bva=bva, bd=bd_eff, qmask=qmask,
                  onesr=np.ones((1, E), np.float32),
                  zerosr=np.zeros((1, SQ), np.float32))
    xTa = np.ascontiguousarray(x.transpose(1, 2, 0))       # (B, E, Sq)
    mTa = np.ascontiguousarray(memory.transpose(1, 2, 0))  # (B, E, Sk)
    in_maps = [dict(shared, xT=xTa[b], mT=mTa[b]) for b in range(B)]
    return in_maps


_NC_CACHE = []


def kernel(x, memory, Wq, bq, Wkv, bkv, Wd, bd):
    from concourse.bass_utils import run_bass_kernel_spmd

    in_maps = host_prep(x, memory, Wq, bq, Wkv, bkv, Wd, bd)
    if not _NC_CACHE:
        _NC_CACHE.append(build_nc())
    nc = _NC_CACHE[0]
    res = run_bass_kernel_spmd(nc, in_maps, core_ids=list(range(B))).results
    out = np.stack([res[b]["out"] for b in range(B)], axis=1)
    return np.ascontiguousarray(out.astype(np.float32))


# revision 22
# speedup vs baseline: 25.6885x; 25.6885x over previous
"""Trainium2 Bass kernel for nn_DeTrCrossAttention (self-contained).

Math (per batch b, derived from the reference's chained permutes):
  64 effective heads indexed by d=0..63, head dim 16 (the h axis).
  Q'[s, d*16+h] = (x_b @ Wq.T + bq)[s, h*64+d]
  K'[s, d*16+h] = (mem_b @ Wkv.T + bkv)[s, h*128+d]
  V'[s, d*16+h] = (mem_b @ Wkv.T)[s, h*128+64+d]      (bv folded into bd_eff)
  S_d = Q'_d @ K'_d.T / 8 ; P_d = softmax(S_d) ; ctx_d = P_d @ V'_d
  out = ctx @ Wd.T + bd_eff,  bd_eff = bd + Wd @ bv'

Sharding: pure data-parallel, one batch element per NeuronCore (B=8, 8 cores).
All matmuls use float32r (full PE stream rate at N>=256, fp32 storage).
Softmax runs in S^T layout [Sk partitions, Sq free]; denominators come from a
ones-column appended to V' (col-tiled M=17 ctx matmuls); no max subtraction
(|S/8| <= ~3 for these input scales, exp is safe in fp32).
"""

import os
import sys

import numpy as np

for _p in ("/opt/trn_rl_repo", os.path.expanduser("~/.axon_site/_ro/trn_rl_repo")):
    if os.path.isdir(_p) and _p not in sys.path:
        sys.path.insert(0, _p)

import concourse.bass as bass
import concourse.tile as tile
from concourse import bacc, mybir

HIDDEN = 1024
NHEAD = 16          # reference NUM_HEADS (becomes the contraction axis)
DH = 64             # reference HEAD_SIZE (becomes the effective head axis)
SQ, SK, B = 256, 1024, 8
E = HIDDEN
ND = DH             # 64 effective heads
HD = NHEAD          # 16 = effective head dim
F32 = mybir.dt.float32
F32R = mybir.dt.float32r


def build_nc():
    nc = bacc.Bacc("TRN2", target_bir_lowering=False)

    xT = nc.dram_tensor("xT", [E, SQ], F32R, kind="ExternalInput")
    mT = nc.dram_tensor("mT", [E, SK], F32R, kind="ExternalInput")
    wqT = nc.dram_tensor("wqT", [E, E], F32R, kind="ExternalInput")
    wkT = nc.dram_tensor("wkT", [E, E], F32R, kind="ExternalInput")
    wvT = nc.dram_tensor("wvT", [E, ND * 17], F32R, kind="ExternalInput")
    wdT = nc.dram_tensor("wdT", [2 * E, E], F32R, kind="ExternalInput")
    bq_d = nc.dram_tensor("bq", [1, E], F32R, kind="ExternalInput")
    bk_d = nc.dram_tensor("bk", [1, E], F32R, kind="ExternalInput")
    bva_d = nc.dram_tensor("bva", [1, ND * 17], F32R, kind="ExternalInput")
    bd_d = nc.dram_tensor("bd", [1, E], F32R, kind="ExternalInput")
    qmask_d = nc.dram_tensor("qmask", [128, 2], F32, kind="ExternalInput")
    ones_d = nc.dram_tensor("onesr", [1, E], F32R, kind="ExternalInput")
    zeros_d = nc.dram_tensor("zerosr", [1, SQ], F32R, kind="ExternalInput")
    out_d = nc.dram_tensor("out", [SQ, E], F32, kind="ExternalOutput")

    VCH = [(0, 384), (384, 384), (768, 320)]  # V'aug N-chunks (all >=256)

    with tile.TileContext(nc) as tc:
        from contextlib import ExitStack

        with ExitStack() as ctx:
            consts = ctx.enter_context(tc.tile_pool(name="consts", bufs=1))
            qab_p = ctx.enter_context(tc.tile_pool(name="qab", bufs=1))
            kt_p = ctx.enter_context(tc.tile_pool(name="ktflat", bufs=1))
            va_p = ctx.enter_context(tc.tile_pool(name="vaug", bufs=1))
            cx_p = ctx.enter_context(tc.tile_pool(name="ctxT", bufs=1))

            ones = consts.tile([1, E], F32R, tag="ones")
            nc.sync.dma_start(ones[:], ones_d[:])
            bq_s = consts.tile([1, E], F32R, tag="bq")
            bk_s = consts.tile([1, E], F32R, tag="bk")
            bva_s = consts.tile([1, ND * 17], F32R, tag="bva")
            bd_s = consts.tile([1, E], F32R, tag="bd")
            qmask_s = consts.tile([128, 2], F32, tag="qmask")
            nc.sync.dma_start(bq_s[:], bq_d[:])
            nc.sync.dma_start(bk_s[:], bk_d[:])
            nc.sync.dma_start(bva_s[:], bva_d[:])
            nc.sync.dma_start(bd_s[:], bd_d[:])
            nc.sync.dma_start(qmask_s[:], qmask_d[:])

            # persistent attention operands
            qa = [qab_p.tile([128, SQ], F32R, tag=f"qa{m}", name=f"qa{m}") for m in range(8)]
            qb = [qab_p.tile([128, SQ], F32R, tag=f"qb{m}", name=f"qb{m}") for m in range(8)]
            kt = [kt_p.tile([128, SK], F32R, tag=f"kt{m}", name=f"kt{m}") for m in range(8)]
            va = [va_p.tile([128, ND * 17], F32R, tag=f"va{m}", name=f"va{m}") for m in range(8)]
            cxT = [cx_p.tile([128, SQ], F32R, tag=f"cx{g}", name=f"cx{g}") for g in range(16)]

            # ---------------- phase 1: projections ----------------
            with ExitStack() as ph1:
                xt_p = ph1.enter_context(tc.tile_pool(name="xt", bufs=1))
                mt_p = ph1.enter_context(tc.tile_pool(name="mt", bufs=1))
                wv_p = ph1.enter_context(tc.tile_pool(name="wv", bufs=1))
                wstr = ph1.enter_context(tc.tile_pool(name="wstr", bufs=4))
                pps = ph1.enter_context(
                    tc.tile_pool(name="pps", bufs=2, space="PSUM")
                )

                xt = [xt_p.tile([128, SQ], F32R, tag=f"xt{k}", name=f"xt{k}") for k in range(8)]
                mt = [mt_p.tile([128, SK], F32R, tag=f"mt{k}", name=f"mt{k}") for k in range(8)]
                for k in range(8):
                    nc.sync.dma_start(xt[k][:], xT[128 * k : 128 * (k + 1), :])
                    nc.sync.dma_start(mt[k][:], mT[128 * k : 128 * (k + 1), :])

                # V'aug = mT.T @ wvT  (+ bias row: only ones-columns nonzero)
                wv = [wv_p.tile([128, ND * 17], F32R, tag=f"wv{k}", name=f"wv{k}") for k in range(8)]
                for k in range(8):
                    nc.sync.dma_start(wv[k][:], wvT[128 * k : 128 * (k + 1), :])
                for mk in range(8):
                    for n0, nw in VCH:
                        ps = pps.tile([128, 384], F32, tag="vps")
                        for k in range(8):
                            nc.tensor.matmul(
                                ps[:, :nw],
                                (mt[k][:, 128 * mk : 128 * (mk + 1)]),
                                (wv[k][:, n0 : n0 + nw]),
                                start=(k == 0),
                                stop=False,
                            )
                        nc.tensor.matmul(
                            ps[:, :nw],
                            (ones[0:1, 0:128]),
                            (bva_s[0:1, n0 : n0 + nw]),
                            start=False,
                            stop=True,
                        )
                        nc.vector.tensor_copy(va[mk][:, n0 : n0 + nw], ps[:, :nw])

                # K'T = wkT.T @ mT   -> kt[m] (flat, rows 16j+h)
                for m in range(8):
                    for n in range(2):
                        ps = pps.tile([128, 512], F32, tag="kps")
                        for k in range(8):
                            wt = wstr.tile([128, 128], F32R, tag="wkt")
                            nc.sync.dma_start(
                                wt[:],
                                wkT[128 * k : 128 * (k + 1), 128 * m : 128 * (m + 1)],
                            )
                            nc.tensor.matmul(
                                ps[:],
                                (wt[:]),
                                (mt[k][:, 512 * n : 512 * (n + 1)]),
                                start=(k == 0),
                                stop=False,
                            )
                        nc.tensor.matmul(
                            ps[:],
                            (bk_s[0:1, 128 * m : 128 * (m + 1)]),
                            (ones[0:1, 0:512]),
                            start=False,
                            stop=True,
                        )
                        nc.vector.tensor_copy(
                            kt[m][:, 512 * n : 512 * (n + 1)], ps[:]
                        )

                # Q'T = wqT.T @ xT -> split even/odd 16-row slices into qa/qb
                for m in range(8):
                    ps = pps.tile([128, SQ], F32, tag="qps")
                    for k in range(8):
                        wt = wstr.tile([128, 128], F32R, tag="wqt")
                        nc.sync.dma_start(
                            wt[:],
                            wqT[128 * k : 128 * (k + 1), 128 * m : 128 * (m + 1)],
                        )
                        nc.tensor.matmul(
                            ps[:], (wt[:]), (xt[k][:]), start=(k == 0), stop=False
                        )
                    nc.tensor.matmul(
                        ps[:],
                        (bq_s[0:1, 128 * m : 128 * (m + 1)]),
                        (ones[0:1, 0:256]),
                        start=False,
                        stop=True,
                    )
                    # full-width copies then per-partition masks (even/odd
                    # 16-row slices; engine APs need 32-aligned bases)
                    nc.vector.tensor_scalar_mul(qa[m][:], ps[:], qmask_s[:, 0:1])
                    nc.vector.tensor_scalar_mul(qb[m][:], ps[:], qmask_s[:, 1:2])

            # ---------------- phase 2: attention ----------------
            with ExitStack() as ph2:
                pt_p = ph2.enter_context(tc.tile_pool(name="pt", bufs=3))
                r_p = ph2.enter_context(tc.tile_pool(name="rt", bufs=3))
                st_p = ph2.enter_context(
                    tc.tile_pool(name="st", bufs=3, space="PSUM")
                )
                cxps = ph2.enter_context(
                    tc.tile_pool(name="cxps", bufs=2, space="PSUM")
                )

                # zero the never-written pad rows of each ctxT slot once
                for g in range(16):
                    for jj in range(4):
                        nc.sync.dma_start(
                            cxT[g][32 * jj + 17 : 32 * jj + 32, :],
                            zeros_d[0:1, :].unsqueeze(1).broadcast_to([1, 15, SQ]),
                        )

                for d in range(ND):
                    g, jj = d // 4, d % 4
                    m, j8 = d // 8, d % 8
                    p = j8 // 2
                    qsrc = qa[m] if j8 % 2 == 0 else qb[m]
                    pt = pt_p.tile([128, 2048], F32R, tag="pt", name="pt")
                    for half in range(2):
                        st = st_p.tile([128, 1024], F32, tag="st", name="st")
                        for i in range(4):
                            kb = 4 * half + i
                            nc.tensor.matmul(
                                st[:, 256 * i : 256 * (i + 1)],
                                kt[m][32 * p : 32 * (p + 1),
                                      128 * kb : 128 * (kb + 1)],
                                qsrc[32 * p : 32 * (p + 1), :],
                                start=True,
                                stop=True,
                                tile_position=(32 * p, 0),
                            )
                        nc.scalar.activation(
                            pt[:, 1024 * half : 1024 * (half + 1)],
                            st[:],
                            mybir.ActivationFunctionType.Exp,
                            scale=0.125,
                        )
                    cps = cxps.tile([17, SQ], F32, tag="cps", name="cps")
                    for kb in range(8):
                        nc.tensor.matmul(
                            cps[:],
                            va[kb][:, 17 * d : 17 * d + 17],
                            pt[:, 256 * kb : 256 * (kb + 1)],
                            start=(kb == 0),
                            stop=(kb == 7),
                        )
                    # unnormalized ctx rows + denominator row -> ctxT slot
                    nc.vector.tensor_copy(cxT[g][32 * jj : 32 * jj + 17, :], cps[:])
                    if jj == 0:
                        rtd = r_p.tile([128, SQ], F32R, tag="rtd", name="rtd")
                        rtr = r_p.tile([128, SQ], F32R, tag="rtr", name="rtr")
                    lo, hi = 32 * jj, 32 * jj + 16
                    nc.sync.dma_start(
                        rtd[lo:hi, :],
                        cxT[g][hi : hi + 1, :]
                        .unsqueeze(1)
                        .broadcast_to([1, 16, SQ]),
                    )
                    with nc.allow_low_precision(reason="f32r recip of softmax denom"):
                        nc.vector.reciprocal(rtr[lo:hi, :], rtd[lo:hi, :])
                    nc.vector.tensor_mul(
                        cxT[g][lo:hi, :],
                        cxT[g][lo:hi, :],
                        rtr[lo:hi, :],
                    )

            # ---------------- phase 3: output projection ----------------
            with ExitStack() as ph3:
                wd_p = ph3.enter_context(tc.tile_pool(name="wd", bufs=1))
                ob_p = ph3.enter_context(tc.tile_pool(name="ob", bufs=2))
                ops = ph3.enter_context(
                    tc.tile_pool(name="ops", bufs=2, space="PSUM")
                )
                wd = [wd_p.tile([128, E], F32R, tag=f"wd{g}", name=f"wd{g}") for g in range(16)]
                for g in range(16):
                    nc.sync.dma_start(wd[g][:], wdT[128 * g : 128 * (g + 1), :])
                for qb_i in range(2):
                    for n in range(2):
                        ps = ops.tile([128, 512], F32, tag="ops")
                        for g in range(16):
                            nc.tensor.matmul(
                                ps[:],
                                (cxT[g][:, 128 * qb_i : 128 * (qb_i + 1)]),
                                (wd[g][:, 512 * n : 512 * (n + 1)]),
                                start=(g == 0),
                                stop=False,
                            )
                        nc.tensor.matmul(
                            ps[:],
                            (ones[0:1, 128 * qb_i : 128 * qb_i + 128]),
                            (bd_s[0:1, 512 * n : 512 * (n + 1)]),
                            start=False,
                            stop=True,
                        )
                        ob = ob_p.tile([128, 512], F32, tag="ob")
                        nc.vector.tensor_copy(ob[:], ps[:])
                        nc.sync.dma_start(
                            out_d[128 * qb_i : 128 * (qb_i + 1),
                                  512 * n : 512 * (n + 1)],
                            ob[:],
                        )

    nc.compile()
    return nc


def host_prep(x, memory, Wq, bq, Wkv, bkv, Wd, bd):
    """Pure-numpy layout transforms shared by all cores + per-core slices."""
    x = np.asarray(x, np.float32)
    memory = np.asarray(memory, np.float32)
    Wq = np.asarray(Wq, np.float32)
    bq = np.asarray(bq, np.float32)
    Wkv = np.asarray(Wkv, np.float32)
    bkv = np.asarray(bkv, np.float32)
    Wd = np.asarray(Wd, np.float32)
    bd = np.asarray(bd, np.float32)

    d_i = np.arange(ND)[:, None]  # 64
    h_i = np.arange(HD)[None, :]  # 16
    perm_q = (h_i * DH + d_i).reshape(-1)          # e' = d*16+h -> h*64+d
    perm_k = (h_i * 2 * DH + d_i).reshape(-1)      # -> h*128+d
    perm_v = (h_i * 2 * DH + DH + d_i).reshape(-1)  # -> h*128+64+d

    wqT = np.ascontiguousarray(Wq[perm_q].T)
    wkT = np.ascontiguousarray(Wkv[perm_k].T)
    bq_p = bq[perm_q].reshape(1, E)
    bk_p = bkv[perm_k].reshape(1, E)
    bv_p = bkv[perm_v]

    wvT = np.zeros((E, ND * 17), np.float32)
    wvT.reshape(E, ND, 17)[:, :, :16] = Wkv[perm_v].T.reshape(E, ND, HD)
    bva = np.zeros((1, ND * 17), np.float32)
    bva.reshape(ND, 17)[:, 16] = 1.0

    wdT = np.zeros((2 * E, E), np.float32)
    wdT.reshape(ND, 32, E)[:, :16, :] = Wd.T.reshape(ND, HD, E)
    bd_eff = (bd + Wd @ bv_p).reshape(1, E)

    qmask = np.zeros((128, 2), np.float32)
    rows = np.arange(128)
    qmask[(rows // 16) % 2 == 0, 0] = 1.0  # even 16-row slices -> qa
    qmask[(rows // 16) % 2 == 1, 1] = 1.0  # odd slices -> qb

    shared = dict(wqT=wqT, wkT=wkT, wvT=wvT, wdT=wdT,
                  bq=bq_p, bk=bk_p, bva=bva, bd=bd_eff, qmask=qmask,
                  onesr=np.ones((1, E), np.float32),
                  zerosr=np.zeros((1, SQ), np.float32))
    xTa = np.ascontiguousarray(x.transpose(1, 2, 0))       # (B, E, Sq)
    mTa = np.ascontiguousarray(memory.transpose(1, 2, 0))  # (B, E, Sk)
    in_maps = [dict(shared, xT=xTa[b], mT=mTa[b]) for b in range(B)]
    return in_maps


_RUN_CACHE = {}


def _get_runner():
    """Build the Bass program once and wrap it in a cached sharded jit.

    Replicates concourse.bass2jax.run_bass_via_pjrt's multi-core path but
    keeps the jitted callable alive so repeat kernel() calls skip program
    rebuild + retrace.
    """
    if "run" in _RUN_CACHE:
        return _RUN_CACHE["run"]
    import jax
    from jax.experimental.shard_map import shard_map
    from jax.sharding import Mesh, PartitionSpec

    from concourse import bass2jax

    bass2jax.install_neuronx_cc_hook()
    nc = build_nc()
    assert nc.dbg_addr is None
    in_names, out_names, out_avals = [], [], []
    for alloc in nc.m.functions[0].allocations:
        if not isinstance(alloc, mybir.MemoryLocationSet):
            continue
        name = alloc.memorylocations[0].name
        if alloc.kind == "ExternalInput":
            in_names.append(name)
        elif alloc.kind == "ExternalOutput":
            out_names.append(name)
            out_avals.append(
                jax.core.ShapedArray(
                    tuple(alloc.tensor_shape), mybir.dt.np(alloc.dtype)
                )
            )
    partition_name = (
        nc.partition_id_tensor.name if nc.partition_id_tensor else None
    )
    if partition_name is not None and partition_name in in_names:
        in_names.remove(partition_name)
    n_params = len(in_names)
    n_outs = len(out_names)
    all_names = in_names + out_names
    if partition_name is not None:
        all_names = all_names + [partition_name]
    out_shapes = [tuple(a.shape) for a in out_avals]

    def _body(*args):
        operands = list(args)
        if partition_name is not None:
            operands.append(bass2jax.partition_id_tensor())
        outs = bass2jax._bass_exec_p.bind(
            *operands,
            out_avals=tuple(out_avals),
            in_names=tuple(all_names),
            out_names=tuple(out_names),
            lowering_input_output_aliases=(),
            sim_require_finite=True,
            sim_require_nnan=True,
            nc=nc,
        )
        return tuple(outs)

    devices = jax.devices()[:B]
    mesh = Mesh(np.asarray(devices), ("core",))
    sharded = jax.jit(
        shard_map(
            _body,
            mesh=mesh,
            in_specs=(PartitionSpec("core"),) * (n_params + n_outs),
            out_specs=(PartitionSpec("core"),) * n_outs,
            check_rep=False,
        ),
        donate_argnums=tuple(range(n_params, n_params + n_outs)),
        keep_unused=True,
    )

    def make_args(in_maps):
        concat_in = [
            np.concatenate([np.asarray(m[nm]) for m in in_maps], axis=0)
            for nm in in_names
        ]
        concat_zeros = [
            np.zeros((B * s[0], *s[1:]), np.float32) for s in out_shapes
        ]
        return concat_in, concat_zeros

    def run(in_maps):
        concat_in, concat_zeros = make_args(in_maps)
        outs = sharded(*concat_in, *concat_zeros)
        return [
            {
                nm: np.asarray(outs[i]).reshape(B, *out_shapes[i])[c]
                for i, nm in enumerate(out_names)
            }
            for c in range(B)
        ]

    _RUN_CACHE["run"] = (run, sharded, make_args, out_shapes)
    return _RUN_CACHE["run"]


def kernel(x, memory, Wq, bq, Wkv, bkv, Wd, bd):
    in_maps = host_prep(x, memory, Wq, bq, Wkv, bkv, Wd, bd)
    run = _get_runner()[0]
    res = run(in_maps)
    out = np.stack([res[b]["out"] for b in range(B)], axis=1)
    return np.ascontiguousarray(out.astype(np.float32))
